# revision 2
# baseline (speedup 1.0000x reference)
"""Trainium2 Bass kernel v3 for the dendritic template-gated FFN.

Math (token n, output feature h; W=16 windows of K=64 input features):
    s[n,h,w] = <x[n, w*64:(w+1)*64], template[h, w*64:(w+1)*64]>
    out[n,h] = sum_w softmax_w(s) * silu(s) = [sum_w silu(s)*e^s] / [sum_w e^s]

v3 structure (vs v2 which ran 2 transcendental ACT passes = 267us ACT busy):
    PE matmuls compute y = A*s directly (A = 2^10*log2(e) folded into xT).
    e^s comes from the Schraudolph exponent-bit trick, NOT from ACT:
        u = int16(y + B).bitcast(fp16)  ~= e^s        (B = 15*2^10 - 48)
    done by DVE tensor_scalar (y psum +B -> int16) or ACT Copy(bias=B),
    split ~5/3 to balance the engines. ACT's single remaining table pass:
        sl = Silu(y * 1/A)                            (exact silu table)
    num elem = sl * u (DVE/Pool TT fp16 2x), reductions on PE:
        den += I@u, num += I@q over the 16 windows; out = num * recip(den).
    Softmax renormalization cancels most of the 3% sawtooth error of the
    fast exp: end-to-end max rel err vs fp64 ~ 4.5e-3 (budget 2e-2).

Engine budget per core (TimelineSim model): PE ~183us (48 mm/region +
transposes), ACT ~183 (8 silu + 3 conv per region), DVE ~176 (5 conv +
4 q + tail + transpose copy-outs), Pool ~136 (4 q per region).

Sharding: data-parallel over tokens, 512 per NeuronCore x 8 cores.
"""

import numpy as np
from contextlib import ExitStack

import concourse.bass as bass
import concourse.bacc as bacc
import concourse.mybir as mybir
import concourse.tile as tile
from concourse.bass_utils import run_bass_kernel_spmd

AF = mybir.ActivationFunctionType
ALU = mybir.AluOpType
DT = mybir.dt

N_TOTAL = 4096
IN_F = 1024
OUT_F = 2048
WIN = 64
NW = 16
N_CORES = 8
N_SH = N_TOTAL // N_CORES   # 512 tokens per core
LAG = 5

A_SCALE = 1477.3197         # 2^10 * log2(e)
B_SHIFT = float(15 * 1024 - 48)   # fp16 exponent bias << 10, Schraudolph shift
INV_A = 1.0 / A_SCALE

# per-group knobs (index g%8): conversion engine and q-product engine
CONV_ON_ACT = (False, True, False, False, True, False, True, False)   # 3/8 on ACT
Q_ON_POOL = (True, False, True, False, True, False, True, False)      # 4/8 on Pool


def build_program(n_tok=N_SH):
    nc = bacc.Bacc(
        "TRN2",
        target_bir_lowering=False,
        debug=False,
        enable_asserts=False,
        num_devices=N_CORES,
    )
    x_d = nc.dram_tensor("x", [n_tok, IN_F], DT.float32, kind="ExternalInput").ap()
    t_d = nc.dram_tensor(
        "template_flat", [OUT_F, IN_F], DT.float32, kind="ExternalInput"
    ).ap()
    eye_d = nc.dram_tensor("eye", [128, 128], DT.float32, kind="ExternalInput").ap()
    out_d = nc.dram_tensor("out", [n_tok, OUT_F], DT.float32, kind="ExternalOutput").ap()

    NT = n_tok // 128       # 4 token tiles
    NJ = OUT_F // 512       # 4 h chunks
    KB = IN_F // 128        # 8 k-blocks (2 windows each)
    HB = OUT_F // 128       # 16 h blocks of template

    with ExitStack() as ctx:
        tc = ctx.enter_context(tile.TileContext(nc))

        const_pool = ctx.enter_context(tc.tile_pool(name="const", bufs=1))
        eye_t = const_pool.tile([128, 128], DT.float32, tag="eye")
        nc.sync.dma_start(eye_t[:], eye_d[:])
        eye_h = const_pool.tile([128, 128], DT.float16, tag="eyeh")
        nc.vector.tensor_copy(eye_h[:], eye_t[:])

        persist = ctx.enter_context(tc.tile_pool(name="persist", bufs=1))
        # xT holds A * x^T (fp16), tT holds t^T (fp16); 2 windows per k-block
        xT = [persist.tile([128, n_tok], DT.float16, tag=f"xT{kb}", name=f"xT{kb}")
              for kb in range(KB)]
        tT = [persist.tile([128, OUT_F], DT.float16, tag=f"tT{kb}", name=f"tT{kb}")
              for kb in range(KB)]

        # staging tiles stay open through the main loop (late transposes)
        stage = ctx.enter_context(tc.tile_pool(name="stage", bufs=1))
        t_nm = [stage.tile([128, IN_F], DT.float32, tag=f"tnm{hb}",
                           name=f"tnm{hb}") for hb in range(HB)]
        x_nm = [stage.tile([128, IN_F], DT.float32, tag=f"xnm{i}",
                           name=f"xnm{i}") for i in range(NT)]
        x_re = x_d.rearrange("(i p) k -> p i k", p=128)
        t_re = t_d.rearrange("(h p) k -> p h k", p=128)
        # region (0,0)'s inputs first; x tile 0 leads because its single
        # DMA feeds the longest transpose+copy chain
        nc.sync.dma_start(x_nm[0][:, 0:512], x_re[:, 0, 0:512])
        nc.sync.dma_start(x_nm[0][:, 512:1024], x_re[:, 0, 512:1024])
        for hb in range(4):
            nc.sync.dma_start(t_nm[hb][:], t_re[:, hb, :])
        for i in range(1, NT):
            nc.sync.dma_start(x_nm[i][:], x_re[:, i, :])
        for hb in range(4, HB):
            nc.sync.dma_start(t_nm[hb][:], t_re[:, hb, :])

        # transpose piece emitters; `ps_fn()` yields a [128,1024] psum tile
        def do_t_chunk(hq, kb, ps_fn):
            """template rows hq*128..(hq+4)*128, k-block kb -> tT[kb]."""
            ps = ps_fn()
            for q in range(4):
                nc.tensor.transpose(
                    ps[:, q * 128:(q + 1) * 128],
                    t_nm[hq + q][:, kb * 128:(kb + 1) * 128],
                    eye_t[:],
                )
            nc.vector.tensor_copy(tT[kb][:, hq * 128:(hq + 4) * 128],
                                  ps[:, 0:512])

        def do_x_piece(i, kb2, ps_fn):
            """x token-tile i, k-blocks kb2..kb2+3 -> xT cols (scaled by A)."""
            ps = ps_fn()
            for kk in range(4):
                kb = kb2 + kk
                nc.tensor.transpose(
                    ps[:, kk * 128:(kk + 1) * 128],
                    x_nm[i][:, kb * 128:(kb + 1) * 128],
                    eye_t[:],
                )
            sl = slice(i * 128, (i + 1) * 128)
            for kk in range(4):
                kb = kb2 + kk
                nc.vector.tensor_scalar(xT[kb][:, sl],
                                        ps[:, kk * 128:(kk + 1) * 128],
                                        A_SCALE, None, ALU.mult)

        # ---- pre-loop: only what region (0,0)'s first groups need ----
        with tc.tile_pool(name="tpsum", bufs=2, space="PSUM") as tpsum:
            def pre_ps():
                return tpsum.tile([128, 1024], DT.float32, tag="tp", name="tp")
            # warm the PE p-state ramp during the input-DMA wait
            warm = tpsum.tile([128, 512], DT.float32, tag="warm", name="warm")
            for _wi in range(25):
                nc.tensor.matmul(warm[:, 0:128], eye_h[:], eye_h[:],
                                 start=True, stop=True,
                                 skip_group_check=True)
            do_x_piece(0, 0, pre_ps)           # xT[:, 0:128] kb 0-3
            do_x_piece(0, 4, pre_ps)           # xT kb 4-7
            do_t_chunk(0, 0, pre_ps)           # tT[0][:, 0:512]
            do_t_chunk(0, 1, pre_ps)
            do_t_chunk(0, 2, pre_ps)
            do_t_chunk(0, 3, pre_ps)

        # ---- main pools ----
        spool = ctx.enter_context(tc.tile_pool(name="spsum", bufs=3, space="PSUM"))
        dnpool = ctx.enter_context(tc.tile_pool(name="dnpsum", bufs=1, space="PSUM"))
        sl_pool = ctx.enter_context(tc.tile_pool(name="slpool", bufs=5))
        iy_pool = ctx.enter_context(tc.tile_pool(name="iypool", bufs=7))
        q_pool = ctx.enter_context(tc.tile_pool(name="qpool", bufs=7))
        tail_pool = ctx.enter_context(tc.tile_pool(name="tail", bufs=2))

        def ring_ps():
            return spool.tile([128, 1024], DT.float32, tag="s", name="s")

        # region-0 just-in-time pieces: tT[g] must be emitted before group
        # g's matmuls (one piece at the top of each early group)
        jit_pieces = [lambda kb=kb: do_t_chunk(0, kb, ring_ps)
                      for kb in range(4, KB)]
        jit_pieces.reverse()

        # late prologue pieces, one per pair of main-loop groups
        late_pieces = []
        for i in range(1, NT):
            for kb2 in (0, 4):
                late_pieces.append(
                    lambda i=i, kb2=kb2: do_x_piece(i, kb2, ring_ps))
        for hq in range(4, HB, 4):
            for kb in range(KB):
                late_pieces.append(
                    lambda hq=hq, kb=kb: do_t_chunk(hq, kb, ring_ps))
        late_pieces.reverse()

        pending = []

        def emit_pending(keep):
            while len(pending) > keep:
                pending.pop(0)()

        for j in range(NJ):
            for i in range(NT):
                r = j * NT + i
                dn = dnpool.tile([128, 1024], DT.float32, tag="dn")
                den = dn[:, 0:512]
                num = dn[:, 512:1024]

                for g in range(8):
                    if jit_pieces:
                        jit_pieces.pop()()
                    # emit deferred reductions BEFORE this group's matmuls so
                    # they aren't FIFO-blocked behind a slot-starved matmul
                    emit_pending(LAG)
                    st = spool.tile([128, 1024], DT.float32, tag="s")
                    for widx in range(2):
                        w = g * 2 + widx
                        base = (w % 2) * 64
                        lhsT = xT[w // 2][base:base + 64,
                                          i * 128:(i + 1) * 128]
                        rhs = tT[w // 2][base:base + 64,
                                         j * 512:(j + 1) * 512]
                        nc.tensor.matmul(
                            st[:, widx * 512:(widx + 1) * 512],
                            lhsT, rhs,
                            start=True, stop=True, skip_group_check=True,
                        )

                    # ACT: exact silu from table, reading y = A*s psum
                    sl_t = sl_pool.tile([128, 1024], DT.float16, tag="sl")
                    nc.scalar.activation(sl_t[:], st[:], AF.Silu,
                                         scale=INV_A)
                    # fast-exp: u = int16(y + B).bitcast(fp16)
                    iy_t = iy_pool.tile([128, 1024], DT.int16, tag="iy")
                    if CONV_ON_ACT[g % 8]:
                        nc.scalar.activation(iy_t[:], st[:], AF.Copy,
                                             bias=B_SHIFT)
                    else:
                        nc.vector.tensor_scalar(iy_t[:], st[:], B_SHIFT,
                                                None, ALU.add)
                    u16 = iy_t[:].bitcast(DT.float16)

                    q_t = q_pool.tile([128, 1024], DT.float16, tag="q")
                    if Q_ON_POOL[g % 8]:
                        nc.gpsimd.tensor_tensor(q_t[:], sl_t[:], u16,
                                                ALU.mult)
                    else:
                        nc.vector.tensor_tensor(q_t[:], sl_t[:], u16,
                                                ALU.mult)

                    def red_task(g=g, den=den, num=num, u16=u16, q_t=q_t):
                        # den mms first so den completes (and the tail's
                        # reciprocal can start) before the num stream ends
                        for c in range(2):
                            w = g * 2 + c
                            sl_ = slice(c * 512, (c + 1) * 512)
                            nc.tensor.matmul(
                                den, eye_h[:], u16[:, sl_],
                                start=(w == 0), stop=(w == NW - 1),
                                skip_group_check=True,
                            )
                        for c in range(2):
                            w = g * 2 + c
                            sl_ = slice(c * 512, (c + 1) * 512)
                            nc.tensor.matmul(
                                num, eye_h[:], q_t[:, sl_],
                                start=(w == 0), stop=(w == NW - 1),
                                skip_group_check=True,
                            )
                    pending.append(red_task)
                    if late_pieces and g % 2 == 1:
                        late_pieces.pop()()

                def tail_task(j=j, i=i, dn=dn, den=den, num=num):
                    r_t = tail_pool.tile([128, 512], DT.float32, tag="r")
                    nc.vector.reciprocal_approx_fast(r_t[:], den)
                    o_t = tail_pool.tile([128, 512], DT.float32, tag="o")
                    nc.vector.tensor_tensor(o_t[:], num, r_t[:], ALU.mult)
                    nc.sync.dma_start(
                        out_d[i * 128:(i + 1) * 128, j * 512:(j + 1) * 512],
                        o_t[:],
                    )
                pending.append(tail_task)
        emit_pending(0)

    nc.compile()
    return nc


_EYE = None
_PROG = None


def _eye_input():
    global _EYE
    if _EYE is None:
        _EYE = np.eye(128, dtype=np.float32)
    return _EYE


def kernel(x: np.ndarray, template_flat: np.ndarray) -> np.ndarray:
    global _PROG
    x = np.ascontiguousarray(x, dtype=np.float32)
    template_flat = np.ascontiguousarray(template_flat, dtype=np.float32)
    assert x.shape == (N_TOTAL, IN_F) and template_flat.shape == (OUT_F, IN_F)
    if _PROG is None:
        _PROG = build_program()
    eye = _eye_input()
    in_maps = [
        {
            "x": x[c * N_SH:(c + 1) * N_SH],
            "template_flat": template_flat,
            "eye": eye,
        }
        for c in range(N_CORES)
    ]
    res = run_bass_kernel_spmd(_PROG, in_maps, core_ids=list(range(N_CORES)))
    return np.concatenate([r["out"] for r in res.results], axis=0)
